# revision 29
# baseline (speedup 1.0000x reference)
"""Multi-head causal attention with RoPE on 8 Trainium2 NeuronCores.

Sharding: core c -> (batch b = c//4, head-group g = c%4, heads 4g..4g+4).
wq/wk/wv column-sharded by head, wo row-sharded; attention fully local.
Host sums the 4 per-core partial output projections per batch.

Numerics: all data fp32; matmuls run in float32r mode (fp32-exact results at
~4x the fp32 streaming rate on TRN2 for free dims >= 256). Softmax without
max-subtraction (scores are O(1); exp cannot overflow). End-to-end rel err
vs the fp32 jax reference ~2e-6.

Layout highlights:
  - all contractions put on the partition axis: q/k computed transposed
    [head_dim, seq] so QK^T, PV and the output projection all chain without
    transposes; scores are built transposed [key, query].
  - RoPE pair-mixing needs a partition swap: the head dim is host-permuted
    (evens then odds per head) so the partner map is a 32-row block swap,
    done with plain SBUF-SBUF DMAs.
  - softmax denominators ride as two extra "ones" columns of V (even heads
    use col 64, odd heads col 65) so each lands on an alignable PSUM row.
  - causal mask = one gpsimd affine_select per score row-tile (diagonal
    128-col block); everything below the diagonal is simply never computed.
"""
import sys
sys.path.insert(0, "/opt/trn_rl_repo")
import numpy as np

import concourse.bass as bass
import concourse.tile as tile
from concourse import bacc, mybir
from concourse.bass_utils import run_bass_kernel_spmd

F = mybir.ActivationFunctionType
A = mybir.AluOpType
FP32 = mybir.dt.float32
FP32R = mybir.dt.float32r
I32 = mybir.dt.int32

B, D, H = 2, 1024, 16
NCORES = 8
GROUPS = 4            # head groups (cores per batch)
HL = H // GROUPS      # heads per core = 4
DK = D // H           # 64
JL = HL * DK          # local projection width = 256
ROPE_THETA = 10000.0

TWO_PI = 2 * np.pi
C1 = 6.28125                      # exact in fp32
C2 = float(np.float32(TWO_PI - C1))
C3 = float(TWO_PI - C1 - C2)
PI = float(np.pi)


def _r(ap):
    """float32r view of an fp32 AP (same bits, fast PE streaming mode)."""
    return ap.bitcast(FP32R)


def build_mha(S: int, max_phase: int = 9, reps: int = 1):
    """One SPMD program: per-core shard of the full MHA layer."""
    assert S % 512 == 0
    NT = S // 128          # 128-tiles along sequence
    NC = S // 512          # 512-chunks along sequence
    KT = D // 128          # contraction tiles for projections

    nc = bacc.Bacc(None, target_bir_lowering=False, debug=False)

    xt_in = nc.declare_dram_parameter("xt", [D, S], FP32, isOutput=False)
    wq_in = nc.declare_dram_parameter("wqt", [D, JL], FP32, isOutput=False)
    wk_in = nc.declare_dram_parameter("wkt", [D, JL], FP32, isOutput=False)
    wv_in = nc.declare_dram_parameter("wvt", [D, JL], FP32, isOutput=False)
    wo_in = nc.declare_dram_parameter("wot", [JL, D], FP32, isOutput=False)
    pos_in = nc.declare_dram_parameter("pos", [1, S], I32, isOutput=False)
    ivf_in = nc.declare_dram_parameter("invfreq", [1, DK], FP32, isOutput=False)
    alt_in = nc.declare_dram_parameter("altsign", [DK, 1], FP32, isOutput=False)
    ind_in = nc.declare_dram_parameter("indicator", [2, 128], FP32, isOutput=False)
    y_out = nc.declare_dram_parameter("y", [S, D], FP32, isOutput=True)

    with tile.TileContext(nc) as tc:
        persist = tc.alloc_tile_pool(name="persist", bufs=1)
        qT = [persist.tile([128, S], FP32R, tag=f"qT{i}", name=f"qT{i}") for i in range(2)]
        kT = [persist.tile([128, S], FP32R, tag=f"kT{i}", name=f"kT{i}") for i in range(2)]
        attnT = [persist.tile([128, S], FP32R, tag=f"aT{i}", name=f"aT{i}") for i in range(2)]
        v_sb = persist.tile([128, NT, HL, DK + 2], FP32R, tag="v")
        cos128 = persist.tile([128, S], FP32R, tag="cos128")
        sinalt128 = persist.tile([128, S], FP32R, tag="sinalt128")

        for _rep in range(reps):
            # ---- Phase 0: trig tables (fp32, once per pass) ----
            with tc.tile_pool(name="trig", bufs=1) as trig, \
                 tc.tile_pool(name="trig_ps", bufs=2, space="PSUM") as trig_ps:
                pos_i = trig.tile([1, S], I32, tag="posi")
                pos_f = trig.tile([1, S], FP32, tag="posf")
                ivf = trig.tile([1, DK], FP32, tag="ivf")
                alt = trig.tile([DK, 1], FP32, tag="alt")
                nc.sync.dma_start(out=pos_i, in_=pos_in[:, :])
                nc.sync.dma_start(out=ivf, in_=ivf_in[:, :])
                nc.sync.dma_start(out=alt, in_=alt_in[:, :])
                nc.vector.tensor_copy(out=pos_f, in_=pos_i)

                ang = trig.tile([DK, S], FP32, tag="ang")
                for c in range(NC):
                    aps = trig_ps.tile([DK, 512], FP32, tag="angps")
                    nc.tensor.matmul(out=aps, lhsT=ivf, rhs=pos_f[:, 512 * c:512 * (c + 1)],
                                     start=True, stop=True)
                    nc.vector.tensor_copy(out=ang[:, 512 * c:512 * (c + 1)], in_=aps)

                # range reduce: xr = ang - round(ang/2pi)*2pi, wrap to [-pi, pi]
                xs = trig.tile([DK, S], FP32, tag="xs")
                ki = trig.tile([DK, S], I32, tag="ki")
                xr = trig.tile([DK, S], FP32, tag="xr")
                kf = xs
                msk = ki.bitcast(FP32)
                nc.vector.tensor_scalar_mul(xs, ang, 1.0 / TWO_PI)
                nc.vector.tensor_copy(out=ki, in_=xs)
                nc.vector.tensor_copy(out=kf, in_=ki)
                nc.vector.scalar_tensor_tensor(xr, kf, -C1, ang, op0=A.mult, op1=A.add)
                nc.vector.scalar_tensor_tensor(xr, kf, -C2, xr, op0=A.mult, op1=A.add)
                nc.vector.scalar_tensor_tensor(xr, kf, -C3, xr, op0=A.mult, op1=A.add)
                nc.vector.tensor_scalar(msk, xr, PI, None, op0=A.is_gt)
                nc.vector.scalar_tensor_tensor(xr, msk, -TWO_PI, xr, op0=A.mult, op1=A.add)

                s64 = trig.tile([DK, S], FP32, tag="s64")
                sR = trig.tile([DK, S], FP32R, tag="sR")
                nc.scalar.activation(out=s64, in_=xr, func=F.Sin)
                nc.vector.tensor_scalar(sR, s64, alt, None, op0=A.mult)
                nc.sync.dma_start(out=sinalt128[0:DK, :], in_=sR)
                nc.sync.dma_start(out=sinalt128[DK:128, :], in_=sR)
                nc.vector.tensor_scalar(xr, xr, PI / 2, None, op0=A.add)
                nc.vector.tensor_scalar(msk, xr, PI, None, op0=A.is_gt)
                nc.vector.scalar_tensor_tensor(xr, msk, -TWO_PI, xr, op0=A.mult, op1=A.add)
                cR = trig.tile([DK, S], FP32R, tag="cR")
                nc.scalar.activation(out=cR, in_=xr, func=F.Sin)
                nc.sync.dma_start(out=cos128[0:DK, :], in_=cR)
                nc.sync.dma_start(out=cos128[DK:128, :], in_=cR)

            # ---- Phase 1+2: projections + RoPE ----
            with tc.tile_pool(name="proj", bufs=1) as proj, \
                 tc.tile_pool(name="wpool", bufs=2) as wpool, \
                 tc.tile_pool(name="ropep", bufs=1) as ropep, \
                 tc.tile_pool(name="proj_ps", bufs=3, space="PSUM") as pps:
                xt = proj.tile([128, KT, S], FP32R, tag="xt")
                for k in range(KT):
                    xstage = wpool.tile([128, S], FP32, tag="stage", name=f"xs{k}")
                    nc.sync.dma_start(out=xstage, in_=xt_in[128 * k:128 * (k + 1), :])
                    nc.vector.tensor_copy(out=xt[:, k, :], in_=xstage)

                # v first (natural layout) so attention can start earliest
                wsv = wpool.tile([128, KT, JL], FP32, tag="stage", name="wsv")
                nc.sync.dma_start(out=wsv, in_=wv_in[:, :].rearrange("(k p) j -> p k j", p=128))
                wv = wpool.tile([128, KT, JL], FP32R, tag="w")
                nc.vector.tensor_copy(out=wv, in_=wsv)
                for st in range(NT):
                    ps = pps.tile([128, JL], FP32, tag="vps")
                    for k in range(KT):
                        nc.tensor.matmul(out=ps, lhsT=(xt[:, k, 128 * st:128 * (st + 1)]),
                                         rhs=(wv[:, k, :]),
                                         start=(k == 0), stop=(k == KT - 1))
                    nc.vector.tensor_copy(
                        out=v_sb[:, st, :, 0:DK],
                        in_=ps[:, :].rearrange("p (h d) -> p h d", h=HL))
                vc32 = wpool.tile([128, NT, HL, 2], FP32, tag="vc32")
                for hh in range(HL):
                    nc.vector.memset(vc32[:, :, hh, 0:1], 1.0 if hh % 2 == 0 else 0.0)
                    nc.vector.memset(vc32[:, :, hh, 1:2], 0.0 if hh % 2 == 0 else 1.0)
                nc.vector.tensor_copy(out=v_sb[:, :, :, DK:DK + 2], in_=vc32)

                # k then q, j-tile 0 then 1; project into the persistent tile,
                # then RoPE in place (swap partner via block DMAs)
                for name, win in (("k", wk_in), ("q", wq_in)):
                    wst = wpool.tile([128, KT, JL], FP32, tag="stage", name=f"ws{name}")
                    nc.sync.dma_start(out=wst, in_=win[:, :].rearrange("(k p) j -> p k j", p=128))
                    wt = wpool.tile([128, KT, JL], FP32R, tag="w", name=f"w{name}")
                    nc.vector.tensor_copy(out=wt, in_=wst)
                    for jt in range(2):
                        t = (kT if name == "k" else qT)[jt]
                        for sc in range(NC):
                            ps = pps.tile([128, 512], FP32, tag="projps")
                            for k in range(KT):
                                nc.tensor.matmul(
                                    out=ps,
                                    lhsT=(wt[:, k, 128 * jt:128 * (jt + 1)]),
                                    rhs=(xt[:, k, 512 * sc:512 * (sc + 1)]),
                                    start=(k == 0), stop=(k == KT - 1))
                            nc.scalar.activation(out=t[:, 512 * sc:512 * (sc + 1)],
                                                 in_=ps, func=F.Copy)
                        # RoPE (perm layout: per 64-row head block, evens then odds)
                        swp = ropep.tile([128, S], FP32R, tag="swp")
                        for blk in range(4):
                            src_b, dst_b = 32 * (blk ^ 1), 32 * blk
                            nc.sync.dma_start(out=swp[dst_b:dst_b + 32, :],
                                              in_=t[src_b:src_b + 32, :])
                        nc.gpsimd.tensor_mul(swp, swp, sinalt128)
                        nc.vector.tensor_mul(t, t, cos128)
                        nc.gpsimd.tensor_add(t, t, swp)

            # ---- Phase 3: attention per head (+ per-jt normalize) ----
            SCALE = 1.0 / np.sqrt(DK)
            late = tc.alloc_tile_pool(name="late", bufs=1)
            woT = late.tile([128, 2, D], FP32R, tag="woT")
            woTs = late.tile([128, 2, D], FP32, tag="woTs")
            nc.sync.dma_start(out=woTs, in_=wo_in[:, :].rearrange("(t p) e -> p t e", p=128))
            nc.vector.tensor_copy(out=woT, in_=woTs)
            with tc.tile_pool(name="attn_es", bufs=6) as es_pool, \
                 tc.tile_pool(name="attn_den", bufs=1) as den_pool, \
                 tc.tile_pool(name="attn_sp", bufs=2, space="PSUM") as sp_pool, \
                 tc.tile_pool(name="attn_ov", bufs=NC, space="PSUM") as ov_pool:
                ind_sb = den_pool.tile([2, 128], FP32, tag="ind")
                nc.sync.dma_start(out=ind_sb, in_=ind_in[:, :])
                den_t = [den_pool.tile([2, S], FP32, tag=f"den{i}", name=f"den{i}")
                         for i in range(2)]
                nc.vector.memset(den_t[0], 0.0)
                nc.vector.memset(den_t[1], 0.0)
                for h in range(HL):
                    jt, pb = h // 2, 64 * (h % 2)
                    kTh = kT[jt]
                    qTh = qT[jt]
                    ov = [ov_pool.tile([DK + 2, 512], FP32, tag="ov", name=f"ov{h}_{i}")
                          for i in range(NC)]

                    def emit_pv(mi, esr):
                        for jg in range(mi // 4, NC):
                            lo = max(512 * jg, 128 * mi)
                            hi = 512 * (jg + 1)
                            nc.tensor.matmul(
                                out=ov[jg][:, lo - 512 * jg:512],
                                lhsT=(v_sb[:, mi, h, :]),
                                rhs=(esr[:, lo - 128 * mi:hi - 128 * mi]),
                                start=(mi == 0), stop=(mi == 4 * jg + 3))

                    pending = None
                    for mi in range(NT):
                        W = S - 128 * mi
                        esr = es_pool.tile([128, S], FP32R, tag="esr")
                        for cb in range(0, W, 1024):
                            cw = min(1024, W - cb)
                            sp = sp_pool.tile([128, 1024], FP32, tag="sp")
                            for sb0 in range(0, cw, 512):
                                sw = min(512, cw - sb0)
                                n0 = 128 * mi + cb + sb0
                                nc.tensor.matmul(
                                    out=sp[:, sb0:sb0 + sw],
                                    lhsT=(kTh[pb:pb + DK, 128 * mi:128 * (mi + 1)]),
                                    rhs=(qTh[pb:pb + DK, n0:n0 + sw]),
                                    start=True, stop=True)
                            nc.scalar.activation(out=esr[:, cb:cb + cw], in_=sp[:, 0:cw],
                                                 func=F.Exp, scale=SCALE)
                        # causal mask on the diagonal 128 cols: keep where n-m >= 0
                        nc.gpsimd.affine_select(
                            out=esr[:, 0:128], in_=esr[:, 0:128],
                            pattern=[[1, 128]], compare_op=A.is_ge, fill=0.0,
                            base=0, channel_multiplier=-1)
                        if pending is not None:
                            emit_pv(*pending)
                        pending = (mi, esr)
                    emit_pv(*pending)
                    # unload: rows 0..63 -> attnT, rows 64..65 -> denominators
                    for jg in range(NC):
                        nc.vector.tensor_copy(
                            out=attnT[jt][pb:pb + DK, 512 * jg:512 * (jg + 1)],
                            in_=ov[jg][0:DK, :])
                        nc.vector.tensor_add(
                            den_t[jt][:, 512 * jg:512 * (jg + 1)],
                            den_t[jt][:, 512 * jg:512 * (jg + 1)],
                            ov[jg][DK:DK + 2, :])
                    if h % 2 == 1:
                        # normalize this j-tile now (bcast via indicator matmul)
                        nc.vector.reciprocal(out=den_t[jt], in_=den_t[jt])
                        for sc in range(NC):
                            bc = ov_pool.tile([128, 512], FP32, tag="ov", name=f"bc{jt}_{sc}")
                            nc.tensor.matmul(out=bc, lhsT=ind_sb,
                                             rhs=den_t[jt][:, 512 * sc:512 * (sc + 1)],
                                             start=True, stop=True)
                            nc.vector.tensor_mul(attnT[jt][:, 512 * sc:512 * (sc + 1)],
                                                 attnT[jt][:, 512 * sc:512 * (sc + 1)], bc)

            # ---- Phase 5: output projection (partial, host reduces) ----
            with tc.tile_pool(name="out_ps", bufs=3, space="PSUM") as ops, \
                 tc.tile_pool(name="out_sb", bufs=4) as osb:
                for st in range(NT):
                    for ec in range(D // 512):
                        po = ops.tile([128, 512], FP32, tag="po")
                        for jt in range(2):
                            nc.tensor.matmul(
                                out=po,
                                lhsT=(attnT[jt][:, 128 * st:128 * (st + 1)]),
                                rhs=(woT[:, jt, 512 * ec:512 * (ec + 1)]),
                                start=(jt == 0), stop=(jt == 1))
                        yst = osb.tile([128, 512], FP32, tag="yst")
                        if ec % 2 == 0:
                            nc.scalar.activation(out=yst, in_=po, func=F.Copy)
                        else:
                            nc.vector.tensor_copy(out=yst, in_=po)
                        nc.sync.dma_start(
                            out=y_out[128 * st:128 * (st + 1), 512 * ec:512 * (ec + 1)],
                            in_=yst)
            late.release()

        persist.release()

    nc.compile()
    return nc


_cache = {}

def _get_program(S):
    if S not in _cache:
        _cache[S] = build_mha(S)
    return _cache[S]


def make_in_maps(x, token_positions, wq, wk, wv, wo):
    S = x.shape[1]
    invfreq = ROPE_THETA ** (-np.arange(0, DK, 2, dtype=np.float32) / DK)
    invfreq_cat = np.concatenate([invfreq, invfreq]).reshape(1, DK).astype(np.float32)
    altsign = np.concatenate([-np.ones(DK // 2), np.ones(DK // 2)]).astype(np.float32).reshape(DK, 1)
    # perm: within each 64-wide head block, evens first then odds
    blockperm = np.concatenate([np.arange(0, DK, 2), np.arange(1, DK, 2)])
    jperm = np.concatenate([64 * hh + blockperm for hh in range(HL)])
    indicator = np.zeros((2, 128), dtype=np.float32)
    indicator[0, 0:64] = 1.0
    indicator[1, 64:128] = 1.0

    in_maps = []
    for c in range(NCORES):
        b, g = c // GROUPS, c % GROUPS
        js = slice(JL * g, JL * (g + 1))
        in_maps.append({
            "xt": np.ascontiguousarray(x[b].T),
            "wqt": np.ascontiguousarray(wq[js, :][jperm, :].T),
            "wkt": np.ascontiguousarray(wk[js, :][jperm, :].T),
            "wvt": np.ascontiguousarray(wv[js, :].T),
            "wot": np.ascontiguousarray(wo[:, js].T),
            "pos": np.asarray(token_positions[b], dtype=np.int32).reshape(1, S),
            "invfreq": invfreq_cat,
            "altsign": altsign,
            "indicator": indicator,
        })
    return in_maps


def kernel(x, token_positions, wq, wk, wv, wo):
    x = np.asarray(x, dtype=np.float32)
    token_positions = np.asarray(token_positions)
    wq = np.asarray(wq, dtype=np.float32)
    wk = np.asarray(wk, dtype=np.float32)
    wv = np.asarray(wv, dtype=np.float32)
    wo = np.asarray(wo, dtype=np.float32)
    S = x.shape[1]

    nc = _get_program(S)
    in_maps = make_in_maps(x, token_positions, wq, wk, wv, wo)
    res = run_bass_kernel_spmd(nc, in_maps, core_ids=list(range(NCORES)))
    out = np.zeros((B, S, D), dtype=np.float32)
    for c in range(NCORES):
        out[c // GROUPS] += res.results[c]["y"]
    return out
